# revision 16
# baseline (speedup 1.0000x reference)
"""ConditionalConv Trainium2 kernel.

Reference computation (B=32, CIN=COUT=32, K=3, H=W=128, COND_DIM=256):
    h = relu(cond @ W1.T + b1)          # [B, 4608]
    w = h @ W2.T + b2                   # [B, 9216] -> per-sample conv weights
    out[b] = conv2d(x[b], w[b])         # 3x3, stride 1, pad 1, per-sample

Distribution over 8 NeuronCores:
  Launch A (hyper-MLP): tensor-shard W2 along its 9216-output dim; every
    core computes h for all B samples (replicated, tiny) and its 1152-wide
    slice of w for all samples.  Host concatenates slices + adds b2.
  Launch B (conv): data-parallel over batch, 4 samples per core.  The four
    samples' [CIN, COUT] weight blocks per 3x3 tap are packed block-diagonally
    into a [128, 128] stationary operand, so each matmul contracts over
    4*CIN = 128 partitions and produces 4*COUT = 128 output channels at once.
    The conv is 9 accumulated matmuls per 512-pixel output tile against a
    host-pre-padded [128, 130, 130] image resident in SBUF.

Precision: matmul operands are fp16 (weights ~N(0, 0.02) and unit-scale
activations fit fp16's 10-bit mantissa; accumulation is fp32 in PSUM) —
plain fp32 matmul would run at 1/4 PE rate and full fp32r (TF32-like) W2
would double the dominant DMA stream.  Measured end-to-end rel-err ~5e-4
vs the fp32 reference.  All DRAM->SBUF transfers are arranged host-side so
each partition reads one contiguous block (small-packet DMA was the
dominant overhead in early profiles).
"""

import numpy as np

import concourse.bass as bass
import concourse.mybir as mybir
import concourse.tile as tile
from concourse import bacc
from concourse.bass_utils import run_bass_kernel_spmd

NCORES = 8
B, CIN, COUT, KK = 32, 32, 32, 3
H = W = 128
COND = 256
NPAR = CIN * COUT * KK * KK   # 9216
HID = NPAR // 2               # 4608
PSH = NPAR // NCORES          # 1152 params per core = 4 couts
BSH = B // NCORES             # 4 samples per core
HCH = HID // 128              # 36 hidden chunks of 128
WGRP = 6                      # stream W2T in 6 groups of 6 chunks
HP, WP = H + 2, W + 2         # padded image

F32 = mybir.dt.float32
F32R = mybir.dt.float32r
F16 = mybir.dt.float16

_cache = {}


def _build_mlp():
    nc = bacc.Bacc(
        "TRN2", target_bir_lowering=False, debug=False, enable_asserts=True,
        num_devices=NCORES,
    )
    condT = nc.dram_tensor("condT", [128, 2, B], F16, kind="ExternalInput").ap()
    w1t = nc.dram_tensor("W1T", [128, 2, HID], F16, kind="ExternalInput").ap()
    b1 = nc.dram_tensor("b1", [128, HCH], F32, kind="ExternalInput").ap()
    w2t = nc.dram_tensor("W2T", [128, HCH, PSH], F16, kind="ExternalInput").ap()
    wout = nc.dram_tensor("wsh", [B, PSH], F32, kind="ExternalOutput").ap()

    with tile.TileContext(nc) as tc:
        with (
            tc.tile_pool(name="consts", bufs=1) as consts,
            tc.tile_pool(name="w2pool", bufs=6) as w2pool,
            tc.tile_pool(name="hpsum", bufs=1, space="PSUM") as hpsum,
            tc.tile_pool(name="wpsum", bufs=1, space="PSUM") as wpsum,
        ):
            condT_sb = consts.tile([128, 2, B], F16, tag="condT")
            w1t_sb = consts.tile([128, 2, HID], F16, tag="w1t")
            b1_sb = consts.tile([128, HCH], F32, tag="b1")
            hT_sb = consts.tile([128, HCH, B], F16, tag="hT")
            w_sb = consts.tile([B, PSH], F32, tag="w")

            nc.sync.dma_start(condT_sb[:], condT)
            nc.sync.dma_start(w1t_sb[:], w1t)
            nc.sync.dma_start(b1_sb[:], b1)

            # ---- MLP1: hT[hid, b] = relu(W1T.T-chunks @ condT + b1) ----
            for g0 in range(0, HCH, 16):
                gn = min(16, HCH - g0)
                ph = hpsum.tile([128, 16, B], F32, tag="ph")
                for j in range(gn):
                    hj = g0 + j
                    for ci in range(2):
                        nc.tensor.matmul(
                            ph[:, j, :],
                            w1t_sb[:, ci, hj * 128:(hj + 1) * 128],
                            condT_sb[:, ci, :],
                            start=(ci == 0),
                            stop=(ci == 1),
                        )
                for j in range(gn):
                    hj = g0 + j
                    # bias (per-partition) + relu + round-to-f32r in one DVE op
                    nc.vector.tensor_scalar(
                        hT_sb[:, hj, :],
                        ph[:, j, :],
                        b1_sb[:, hj:hj + 1],
                        0.0,
                        mybir.AluOpType.add,
                        mybir.AluOpType.max,
                    )

            # ---- MLP2: w[b, p] = hT.T-chunks @ W2T-chunks ----
            pw0 = wpsum.tile([B, 512], F32, tag="pw0")
            pw1 = wpsum.tile([B, 512], F32, tag="pw1")
            pw2 = wpsum.tile([B, 128], F32, tag="pw2")
            pws = [(pw0, 0, 512), (pw1, 512, 512), (pw2, 1024, 128)]
            for g in range(WGRP):
                w2g = w2pool.tile([128, HCH // WGRP, PSH], F16, tag="w2g")
                nc.sync.dma_start(
                    w2g[:], w2t[:, g * (HCH // WGRP):(g + 1) * (HCH // WGRP), :]
                )
                for j in range(HCH // WGRP):
                    hj = g * (HCH // WGRP) + j
                    for pt, p0, pn in pws:
                        nc.tensor.matmul(
                            pt[:, :pn],
                            hT_sb[:, hj, :],
                            w2g[:, j, p0:p0 + pn],
                            start=(hj == 0),
                            stop=(hj == HCH - 1),
                        )
            for pt, p0, pn in pws:
                nc.vector.tensor_copy(w_sb[:, p0:p0 + pn], pt[:, :pn])
            nc.sync.dma_start(wout, w_sb[:])
    nc.compile()
    return nc


def _build_conv():
    nc = bacc.Bacc(
        "TRN2", target_bir_lowering=False, debug=False, enable_asserts=True,
        num_devices=NCORES,
    )
    # x arrives host-pre-padded: [BSH, CIN, 130, 130] with zero borders
    xs = nc.dram_tensor("xs", [BSH, CIN, HP, WP], F16, kind="ExternalInput").ap()
    wst = nc.dram_tensor("wst", [9, 128, 128], F16, kind="ExternalInput").ap()
    ys = nc.dram_tensor("ys", [BSH, COUT, H, W], F32, kind="ExternalOutput").ap()

    xv = xs.rearrange("s c h w -> (s c) h w")   # [128, 130, 130]
    yv = ys.rearrange("s c h w -> (s c) h w")

    with tile.TileContext(nc) as tc:
        with (
            tc.tile_pool(name="sb", bufs=1) as sb,
            tc.tile_pool(name="outp", bufs=4) as outp,
            tc.tile_pool(name="cpsum", bufs=8, space="PSUM") as cpsum,
        ):
            wst_sb = sb.tile([128, 9, 128], F16, tag="wst")
            nc.sync.dma_start(wst_sb[:], wst.rearrange("t k m -> k t m"))

            xp = sb.tile([128, HP, WP], F16, tag="xp")
            # row chunks; each partition reads contiguous bytes per chunk.
            # finer chunks at the head let the first matmuls start sooner
            bounds = [0, 7, 14, 26, 52, 78, 104, 130]
            for a, b in zip(bounds[:-1], bounds[1:]):
                nc.sync.dma_start(xp[:, a:b, :], xv[:, a:b, :])

            for r0 in range(H // 4):
                ps = cpsum.tile([128, 4, W], F32, tag="cp")
                for t in range(9):
                    kh, kw = divmod(t, 3)
                    nc.tensor.matmul(
                        ps[:],
                        wst_sb[:, t, :],
                        xp[:, r0 * 4 + kh:r0 * 4 + kh + 4, kw:kw + W],
                        start=(t == 0),
                        stop=(t == 8),
                    )
                ot = outp.tile([128, 4, W], F32, tag="ot")
                if r0 % 3 == 2:
                    nc.scalar.activation(
                        ot[:], ps[:], mybir.ActivationFunctionType.Copy
                    )
                else:
                    nc.vector.tensor_copy(ot[:], ps[:])
                nc.gpsimd.dma_start(yv[:, r0 * 4:(r0 + 1) * 4, :], ot[:])
    nc.compile()
    return nc


def _get_programs():
    if "mlp" not in _cache:
        _cache["mlp"] = _build_mlp()
    if "conv" not in _cache:
        _cache["conv"] = _build_conv()
    return _cache["mlp"], _cache["conv"]


def kernel(x, cond, W1, b1, W2, b2, _trace=False):
    x = np.ascontiguousarray(np.asarray(x, dtype=np.float32))
    cond = np.asarray(cond, dtype=np.float32)
    W1 = np.asarray(W1, dtype=np.float32)
    b1 = np.asarray(b1, dtype=np.float32)
    W2 = np.asarray(W2, dtype=np.float32)
    b2 = np.asarray(b2, dtype=np.float32)

    nc_mlp, nc_conv = _get_programs()
    core_ids = list(range(NCORES))

    # host-side layout prep: every SBUF destination gets one contiguous
    # per-partition read
    condTS = np.ascontiguousarray(
        cond.T.reshape(2, 128, B).transpose(1, 0, 2)
    ).astype(np.float16)
    W1TS = np.ascontiguousarray(
        W1.T.reshape(2, 128, HID).transpose(1, 0, 2)
    ).astype(np.float16)
    b1S = np.ascontiguousarray(b1.reshape(HCH, 128).T)
    # [8, 128, 36, 1152]: per-core pre-transposed W2 shard
    W2TS = np.ascontiguousarray(
        W2.T.reshape(HCH, 128, NCORES, PSH).transpose(2, 1, 0, 3)
    ).astype(np.float16)

    in_maps_a = [
        {"condT": condTS, "W1T": W1TS, "b1": b1S, "W2T": W2TS[i]}
        for i in core_ids
    ]
    res_a = run_bass_kernel_spmd(nc_mlp, in_maps_a, core_ids, trace=_trace)

    w = np.concatenate([res_a.results[i]["wsh"] for i in core_ids], axis=1)
    w = w + b2[None, :]                      # [B, 9216]
    wr = w.reshape(B, COUT, CIN, 9)

    xpad = np.zeros((B, CIN, HP, WP), dtype=np.float16)
    xpad[:, :, 1:H + 1, 1:W + 1] = x

    in_maps_b = []
    for i in core_ids:
        blk = np.zeros((9, 128, 128), dtype=np.float16)
        for s in range(BSH):
            # [t, cin, cout] block for sample 4i+s on the diagonal
            blk[:, s * CIN:(s + 1) * CIN, s * COUT:(s + 1) * COUT] = (
                wr[i * BSH + s].transpose(2, 1, 0)
            )
        in_maps_b.append({"xs": xpad[i * BSH:(i + 1) * BSH], "wst": blk})
    res_b = run_bass_kernel_spmd(nc_conv, in_maps_b, core_ids, trace=_trace)

    out = np.concatenate([res_b.results[i]["ys"] for i in core_ids], axis=0)
    if _trace:
        return out, (res_a, res_b)
    return out
